# revision 1
# baseline (speedup 1.0000x reference)
"""Trainium2 Bass kernel for nn_CounterFlowNetwork.

Data-parallel over 8 NeuronCores (batch sharded), with the whole
counterflow sweep restructured to minimize matmul and elementwise work:

 - Consecutive linear layers folded host-side:
     delta @ W_ab -> liquid tracked in "equilibrium-projected" space
     (W_trabeq = alpha*W_tr @ W_ab @ W_eq), so a descending plate is ONE
     256x256 matmul instead of three.
 - The descending-sweep sigmoid at plate n-1 and the ascending-sweep
   sigmoid at plate n use the *same* l[n], so each sweep needs only 8
   sigmoid field evals instead of 16.
 - l[1] for the output head is recovered from S = sum of descending
   driving forces:  l1 @ W1_l = S @ (W_trab @ W1_l) + const.
 - All activations live transposed in SBUF ([feature, row]); the final
   head matmul uses h as the stationary operand so the output lands in
   natural [row, feature] layout for a clean DMA out.
 - Matmuls run in float32r (tf32-like, full PE rate); biases are folded
   into ACT activation biases or injected with K=1 ones-matmuls.
"""

import numpy as np

import concourse.bass as bass
import concourse.bacc as bacc
import concourse.mybir as mybir
import concourse.tile as tile
from concourse import bass_utils

B, D_IN, D_GAS, D_OUT = 16384, 512, 256, 1000
N_PLATES = 8
N_CORES = 8
ROWS = B // N_CORES          # rows per core
N_CHUNKS = 4
R = ROWS // N_CHUNKS         # rows per chunk
F32 = mybir.dt.float32
F32R = mybir.dt.float32r
AF = mybir.ActivationFunctionType
OP = mybir.AluOpType

# How many descending-df evals (out of 16 per chunk) go to GPSIMD
DESC_DF_ON_GPSIMD = {(0, 4), (0, 5), (1, 4)}  # (sweep, plate) pairs


def _preprocess_weights(inp):
    """Fold the plate linear algebra host-side (float64, cast to f32)."""
    f32, f64 = np.float32, np.float64
    W_tr = np.asarray(inp["W_tr"], f32)
    b_tr = np.asarray(inp["b_tr"], f32)
    W_ab = np.asarray(inp["W_ab"], f32)
    b_ab = np.asarray(inp["b_ab"], f32)
    W_eq = np.asarray(inp["W_eq"], f32)
    b_eq = np.asarray(inp["b_eq"], f32)
    W1 = np.asarray(inp["W1"], f32)
    b1 = np.asarray(inp["b1"], f32)
    alpha = float(np.asarray(inp["alpha"]))

    W_trp = (alpha * W_tr).astype(f32)
    ab_tr = (alpha * b_tr).astype(f32)
    W_trab = (W_trp.astype(f64) @ W_ab).astype(f32)
    c2 = (ab_tr.astype(f64) @ W_ab + b_ab).astype(f32)
    W_trabeq = (W_trab.astype(f64) @ W_eq).astype(f32)
    c3 = (c2.astype(f64) @ W_eq).astype(f32)
    W1_g, W1_l = W1[:D_GAS], W1[D_GAS:]
    W_fold = (W_trab.astype(f64) @ W1_l).astype(f32)
    bias_e = np.stack([(b_eq + (9 - n) * c3) for n in range(1, 9)]).astype(f32)
    e9 = (1.0 / (1.0 + np.exp(-b_eq.astype(f64)))).astype(f32)
    h_bias = (b1 + 8.0 * (c2.astype(f64) @ W1_l)).astype(f32)

    return {
        "wge": np.ascontiguousarray(np.asarray(inp["W_ge"], f32)),
        "wdesc": np.ascontiguousarray(W_trabeq),
        "wasc": np.ascontiguousarray(-W_trp),
        "wfold": np.ascontiguousarray(W_fold),
        "w1g": np.ascontiguousarray(W1_g),
        "w2": np.ascontiguousarray(np.asarray(inp["W2"], f32)),
        "nab": np.ascontiguousarray(-ab_tr.reshape(1, D_GAS)),
        "b2r": np.ascontiguousarray(np.asarray(inp["b2"], f32).reshape(1, D_OUT)),
        "be": np.ascontiguousarray(bias_e),          # (8, 256), index n-1
        "ne9": np.ascontiguousarray(-e9),            # (256,)
        "bge": np.ascontiguousarray(np.asarray(inp["b_ge"], f32)),
        "hb": np.ascontiguousarray(h_bias),
        "iden": np.eye(128, dtype=f32),
        "ones": np.ones((1, R), f32),
    }


def build_nc():
    nc = bacc.Bacc("TRN2", target_bir_lowering=False, debug=False)

    x_d = nc.dram_tensor("x", (ROWS, D_IN), F32R, kind="ExternalInput").ap()
    wge_d = nc.dram_tensor("wge", (D_IN, D_GAS), F32R, kind="ExternalInput").ap()
    wdesc_d = nc.dram_tensor("wdesc", (D_GAS, D_GAS), F32R, kind="ExternalInput").ap()
    wasc_d = nc.dram_tensor("wasc", (D_GAS, D_GAS), F32R, kind="ExternalInput").ap()
    wfold_d = nc.dram_tensor("wfold", (D_GAS, D_GAS), F32R, kind="ExternalInput").ap()
    w1g_d = nc.dram_tensor("w1g", (D_GAS, D_GAS), F32R, kind="ExternalInput").ap()
    w2_d = nc.dram_tensor("w2", (D_GAS, D_OUT), F32R, kind="ExternalInput").ap()
    nab_d = nc.dram_tensor("nab", (1, D_GAS), F32R, kind="ExternalInput").ap()
    b2r_d = nc.dram_tensor("b2r", (1, D_OUT), F32R, kind="ExternalInput").ap()
    be_d = nc.dram_tensor("be", (8, D_GAS), F32, kind="ExternalInput").ap()
    ne9_d = nc.dram_tensor("ne9", (D_GAS,), F32, kind="ExternalInput").ap()
    bge_d = nc.dram_tensor("bge", (D_GAS,), F32, kind="ExternalInput").ap()
    hb_d = nc.dram_tensor("hb", (D_GAS,), F32, kind="ExternalInput").ap()
    iden_d = nc.dram_tensor("iden", (128, 128), F32R, kind="ExternalInput").ap()
    ones_d = nc.dram_tensor("ones", (1, R), F32R, kind="ExternalInput").ap()
    out_d = nc.dram_tensor("out", (ROWS, D_OUT), F32, kind="ExternalOutput").ap()

    with tile.TileContext(nc) as tc:
        with (
            tc.tile_pool(name="const", bufs=1) as cpool,
            tc.tile_pool(name="state", bufs=2) as spool,
            tc.tile_pool(name="work", bufs=3) as wpool,
            tc.tile_pool(name="psum", bufs=1, space="PSUM") as ppool,
        ):
            # ---- constants ----
            wge_t = cpool.tile([128, 4, D_GAS], F32R, tag="wge")
            nc.sync.dma_start(wge_t, wge_d.rearrange("(ko ki) m -> ki ko m", ki=128))
            wdesc_t = cpool.tile([128, 2, D_GAS], F32R, tag="wdesc")
            nc.sync.dma_start(wdesc_t, wdesc_d.rearrange("(ko ki) m -> ki ko m", ki=128))
            wasc_t = cpool.tile([128, 2, D_GAS], F32R, tag="wasc")
            nc.sync.dma_start(wasc_t, wasc_d.rearrange("(ko ki) m -> ki ko m", ki=128))
            wfold_t = cpool.tile([128, 2, D_GAS], F32R, tag="wfold")
            nc.sync.dma_start(wfold_t, wfold_d.rearrange("(ko ki) m -> ki ko m", ki=128))
            w1g_t = cpool.tile([128, 2, D_GAS], F32R, tag="w1g")
            nc.sync.dma_start(w1g_t, w1g_d.rearrange("(ko ki) m -> ki ko m", ki=128))
            w2_t = cpool.tile([128, 2, D_OUT], F32R, tag="w2")
            nc.sync.dma_start(w2_t, w2_d.rearrange("(ko ki) n -> ki ko n", ki=128))
            nab_t = cpool.tile([1, D_GAS], F32R, tag="nab")
            nc.sync.dma_start(nab_t, nab_d)
            b2r_t = cpool.tile([1, D_OUT], F32R, tag="b2r")
            nc.sync.dma_start(b2r_t, b2r_d)
            be_t = cpool.tile([128, 8, 2], F32, tag="be")
            nc.sync.dma_start(be_t, be_d.rearrange("n (f k) -> k n f", k=128))
            ne9_t = cpool.tile([128, 2], F32, tag="ne9")
            nc.sync.dma_start(ne9_t, ne9_d.rearrange("(f k) -> k f", k=128))
            bge_t = cpool.tile([128, 2], F32, tag="bge")
            nc.sync.dma_start(bge_t, bge_d.rearrange("(f k) -> k f", k=128))
            hb_t = cpool.tile([128, 2], F32, tag="hb")
            nc.sync.dma_start(hb_t, hb_d.rearrange("(f k) -> k f", k=128))
            iden_t = cpool.tile([128, 128], F32R, tag="iden")
            nc.sync.dma_start(iden_t, iden_d)
            ones_t = cpool.tile([1, R], F32R, tag="ones")
            nc.sync.dma_start(ones_t, ones_d)

            for c in range(N_CHUNKS):
                par = c % 2
                r0 = c * R

                # ---- load x and transpose into [feature, row] layout ----
                xnat = []
                for rb in range(R // 128):
                    xn = wpool.tile([128, D_IN], F32R, tag="xnat", bufs=5)
                    nc.sync.dma_start(xn, x_d[r0 + rb * 128 : r0 + (rb + 1) * 128, :])
                    xnat.append(xn)
                xT = wpool.tile([128, 4, R], F32R, tag="xT", bufs=2)
                for k in range(4):
                    pt = ppool.tile([128, R], F32R, tag="pt", bufs=2)
                    for rb in range(R // 128):
                        nc.tensor.transpose(
                            pt[:, rb * 128 : (rb + 1) * 128],
                            xnat[rb][:, k * 128 : (k + 1) * 128],
                            iden_t,
                        )
                    nc.vector.tensor_copy(xT[:, k, :], pt)

                # ---- encoder: g0 = relu(x @ W_ge + b_ge), transposed ----
                p_enc = [ppool.tile([128, R], F32, tag=f"sweep{par}f{ft}", bufs=1, name=f"penc{c}_{ft}")
                         for ft in range(2)]
                for ft in range(2):
                    for k in range(4):
                        nc.tensor.matmul(
                            p_enc[ft],
                            lhsT=wge_t[:, k, ft * 128 : (ft + 1) * 128],
                            rhs=xT[:, k, :],
                            start=(k == 0),
                            stop=(k == 3),
                        )
                g0 = spool.tile([128, 2, R], F32R, tag="g0")
                for ft in range(2):
                    nc.scalar.activation(
                        g0[:, ft, :], p_enc[ft], AF.Relu,
                        bias=bge_t[:, ft : ft + 1],
                    )

                st = {}      # plate index -> current SBUF tile (e or g)
                S = None
                for sweep in range(2):
                    last = sweep == 1
                    # ---------- descending sweep ----------
                    pacc = wpool.tile([128, 2, R], F32, tag="pacc", bufs=2)
                    for n in range(N_PLATES, 0, -1):
                        df = wpool.tile([128, 2, R], F32R, tag="df", bufs=3)
                        g_prev = g0 if (sweep == 0 or n == 1) else st[n - 1]
                        if n == N_PLATES:
                            for ft in range(2):
                                nc.vector.tensor_scalar(
                                    df[:, ft, :], g_prev[:, ft, :],
                                    ne9_t[:, ft : ft + 1], None, OP.add,
                                )
                        else:
                            eng = (
                                nc.gpsimd
                                if (sweep, n) in DESC_DF_ON_GPSIMD
                                else nc.vector
                            )
                            eng.tensor_tensor(df, g_prev, st[n + 1], OP.subtract)
                        if last:
                            if n == N_PLATES:
                                S = spool.tile([128, 2, R], F32, tag="S")
                                nc.gpsimd.tensor_copy(S, df)
                            else:
                                nc.gpsimd.tensor_tensor(S, S, df, OP.add)
                        pd = [ppool.tile([128, R], F32, tag=f"sweep{par}f{ft}", bufs=1, name=f"pd{c}_{sweep}_{n}_{ft}")
                              for ft in range(2)]
                        for ft in range(2):
                            for k in range(2):
                                nc.tensor.matmul(
                                    pd[ft],
                                    lhsT=wdesc_t[:, k, ft * 128 : (ft + 1) * 128],
                                    rhs=df[:, k, :],
                                    start=(k == 0),
                                    stop=(k == 1),
                                )
                        e_new = spool.tile([128, 2, R], F32R, tag=f"st{n}")
                        for ft in range(2):
                            if n == N_PLATES:
                                nc.vector.tensor_copy(pacc[:, ft, :], pd[ft])
                            else:
                                nc.vector.tensor_tensor(
                                    pacc[:, ft, :], pacc[:, ft, :], pd[ft], OP.add
                                )
                            nc.scalar.activation(
                                e_new[:, ft, :], pacc[:, ft, :], AF.Sigmoid,
                                bias=be_t[:, n - 1, ft : ft + 1],
                            )
                        st[n] = e_new

                    # ---------- ascending sweep ----------
                    g_acc = wpool.tile([128, 2, R], F32, tag="gacc", bufs=2)
                    nc.vector.tensor_copy(g_acc, g0)
                    for n in range(1, N_PLATES + 1):
                        df = wpool.tile([128, 2, R], F32R, tag="df", bufs=3)
                        nc.vector.tensor_tensor(df, g_acc, st[n], OP.subtract)
                        pd = [ppool.tile([128, R], F32, tag=f"sweep{par}f{ft}", bufs=1, name=f"pa{c}_{sweep}_{n}_{ft}")
                              for ft in range(2)]
                        for ft in range(2):
                            for k in range(2):
                                nc.tensor.matmul(
                                    pd[ft],
                                    lhsT=wasc_t[:, k, ft * 128 : (ft + 1) * 128],
                                    rhs=df[:, k, :],
                                    start=(k == 0), stop=False,
                                )
                            nc.tensor.matmul(
                                pd[ft],
                                lhsT=nab_t[0:1, ft * 128 : (ft + 1) * 128],
                                rhs=ones_t[0:1, :],
                                start=False, stop=True,
                            )
                            nc.vector.tensor_tensor(
                                g_acc[:, ft, :], g_acc[:, ft, :], pd[ft], OP.add
                            )
                        if (not last and n <= N_PLATES - 1) or (last and n == N_PLATES):
                            g_sn = spool.tile([128, 2, R], F32R, tag=f"st{n}")
                            nc.scalar.copy(g_sn, g_acc)
                            st[n] = g_sn

                # ---------- head ----------
                S_r = wpool.tile([128, 2, R], F32R, tag="S_r", bufs=2)
                nc.vector.tensor_copy(S_r, S)
                g8 = st[N_PLATES]
                p_h = [ppool.tile([128, R], F32, tag=f"sweep{par}f{ft}", bufs=1, name=f"ph{c}_{ft}")
                       for ft in range(2)]
                for ft in range(2):
                    for k in range(2):
                        nc.tensor.matmul(
                            p_h[ft],
                            lhsT=w1g_t[:, k, ft * 128 : (ft + 1) * 128],
                            rhs=g8[:, k, :],
                            start=(k == 0), stop=False,
                        )
                    for k in range(2):
                        nc.tensor.matmul(
                            p_h[ft],
                            lhsT=wfold_t[:, k, ft * 128 : (ft + 1) * 128],
                            rhs=S_r[:, k, :],
                            start=False, stop=(k == 1),
                        )
                h = wpool.tile([128, 2, R], F32R, tag="h", bufs=2)
                for ft in range(2):
                    nc.scalar.activation(
                        h[:, ft, :], p_h[ft], AF.Relu,
                        bias=hb_t[:, ft : ft + 1],
                    )
                # out = h @ W2 + b2, with h as the stationary operand so the
                # result lands natural [row, feature]
                for rb in range(R // 128):
                    p_o = ppool.tile([128, D_OUT], F32, tag="po", bufs=1)
                    for n0, nw in ((0, 512), (512, 488)):
                        for ft in range(2):
                            nc.tensor.matmul(
                                p_o[:, n0 : n0 + nw],
                                lhsT=h[:, ft, rb * 128 : (rb + 1) * 128],
                                rhs=w2_t[:, ft, n0 : n0 + nw],
                                start=(ft == 0), stop=False,
                            )
                        nc.tensor.matmul(
                            p_o[:, n0 : n0 + nw],
                            lhsT=ones_t[0:1, 0:128],
                            rhs=b2r_t[0:1, n0 : n0 + nw],
                            start=False, stop=True,
                        )
                    stage = wpool.tile([128, D_OUT], F32, tag="stage", bufs=3)
                    nc.vector.tensor_copy(stage, p_o)
                    nc.sync.dma_start(
                        out_d[r0 + rb * 128 : r0 + (rb + 1) * 128, :], stage
                    )

    nc.compile()
    return nc


_NC_CACHE = {}


def kernel(**inputs):
    inp = {k: np.asarray(v) for k, v in inputs.items()}
    prep = _preprocess_weights(inp)
    x = np.ascontiguousarray(inp["x"], dtype=np.float32)

    if "nc" not in _NC_CACHE:
        _NC_CACHE["nc"] = build_nc()
    nc = _NC_CACHE["nc"]

    in_maps = []
    for c in range(N_CORES):
        m = {"x": x[c * ROWS : (c + 1) * ROWS]}
        m.update(prep)
        in_maps.append(m)
    res = bass_utils.run_bass_kernel_spmd(nc, in_maps, core_ids=list(range(N_CORES)))
    out = np.concatenate([res.results[c]["out"] for c in range(N_CORES)], axis=0)
    return out



# revision 5
# speedup vs baseline: 1.8865x; 1.8865x over previous
"""Trainium2 Bass kernel for nn_CounterFlowNetwork.

Data-parallel over 8 NeuronCores (batch sharded).  v2 restructure vs the
f32r baseline (772us):

 - bf16 activations + weights everywhere (PE rate identical, DVE gets the
   2x packed fast path, SBUF traffic halves).  Tolerance is 2e-2; bf16
   plate chain lands ~1e-3.
 - x is cast to bf16 host-side and loaded TRANSPOSED by the DMA xbar
   (dma_start_transpose), removing all PE transposes / cast ops.
 - The descending-sweep accumulator pacc lives in PSUM: plate matmuls
   accumulate with start=False instead of DVE adds, and the sigmoid
   reads PSUM directly.  Per-plate biases (c3 increments, b_eq, -alpha
   b_tr) are injected with K=1 ones-matmuls so the activations need no
   per-partition bias and can process both 128-feature halves in ONE
   fused [128,1024] op.
 - Ascending sweep 0 materializes g_n into SBUF state tiles (needed by
   the next descending sweep), alternating DVE-add / identity-matmul +
   ACT-copy to balance engines.  Ascending sweep 1 keeps g in a PSUM
   accumulator (nobody needs g_1..g_7 of the last sweep in SBUF) and
   only copies g_8 out for the head.  Plate 8 of sweep 0 is dead code
   (reference discards it) and is skipped.
 - l[1] for the head is recovered from S = sum of final-sweep descending
   driving forces (GPSIMD accumulates), as in v1.
 - Output bias b2 is added host-side after the gather, removing the
   N=1000 bias matmuls; out = h @ W2 DMAs straight from PSUM.
 - Work for chunk pairs (0,1) and (2,3) is issue-interleaved via
   generators so each engine's in-order queue always has independent
   work from the sibling chunk -> PE stays busy and ramps to full clock.
"""

import numpy as np

import concourse.bass as bass
import concourse.bacc as bacc
import concourse.mybir as mybir
import concourse.tile as tile
from concourse import bass_utils

B, D_IN, D_GAS, D_OUT = 16384, 512, 256, 1000
N_PLATES = 8
N_CORES = 8
ROWS = B // N_CORES          # rows per core
N_CHUNKS = 4
R = ROWS // N_CHUNKS         # rows per chunk
F32 = mybir.dt.float32
BF16 = mybir.dt.bfloat16
NPBF16 = mybir.dt.np(BF16)
AF = mybir.ActivationFunctionType
OP = mybir.AluOpType

# ascending sweep 0: which plates materialize g via iden-matmul + ACT copy
# (rest use a DVE add); ascending sweep 1: which plate dfs go to GPSIMD
ASC0_ACT_PLATES = {2, 4, 6}
ASC1_GPSIMD_PLATES = {2, 5, 7}


def _preprocess_weights(inp):
    """Fold the plate linear algebra host-side (float64, cast to bf16)."""
    f32, f64 = np.float32, np.float64
    W_tr = np.asarray(inp["W_tr"], f32)
    b_tr = np.asarray(inp["b_tr"], f32)
    W_ab = np.asarray(inp["W_ab"], f32)
    b_ab = np.asarray(inp["b_ab"], f32)
    W_eq = np.asarray(inp["W_eq"], f32)
    b_eq = np.asarray(inp["b_eq"], f32)
    W1 = np.asarray(inp["W1"], f32)
    b1 = np.asarray(inp["b1"], f32)
    alpha = float(np.asarray(inp["alpha"]))

    W_trp = alpha * W_tr.astype(f64)
    ab_tr = alpha * b_tr.astype(f64)
    W_trab = W_trp @ W_ab
    c2 = ab_tr @ W_ab + b_ab
    W_trabeq = W_trab @ W_eq
    c3 = c2 @ W_eq
    W1_g, W1_l = W1[:D_GAS].astype(f64), W1[D_GAS:].astype(f64)
    W_fold = W_trab @ W1_l
    e9 = 1.0 / (1.0 + np.exp(-b_eq.astype(f64)))
    h_bias = b1 + 8.0 * (c2 @ W1_l)

    def kmaj(w):  # (K, M) -> [128, K//128, M] partition-major contraction
        w = np.asarray(w, NPBF16)
        k, m = w.shape
        return np.ascontiguousarray(w.reshape(k // 128, 128, m).transpose(1, 0, 2))

    return {
        "wge": kmaj(np.asarray(inp["W_ge"], f32)),
        "wdesc": kmaj(W_trabeq),
        "wasc": kmaj(-W_trp),
        "wfold": kmaj(W_fold),
        "w1g": kmaj(W1_g),
        "w2": kmaj(np.asarray(inp["W2"], f32)),
        "iden": np.eye(128, dtype=np.float32).astype(NPBF16),
        "ones": np.ones((1, R), NPBF16),
        # K=1 bias rows for ones-matmul injection
        "bge_r": np.asarray(b_eq * 0 + np.asarray(inp["b_ge"], f32), NPBF16).reshape(1, D_GAS),
        "be8_r": np.asarray(b_eq + c3, NPBF16).reshape(1, D_GAS),
        "c3_r": np.asarray(c3, NPBF16).reshape(1, D_GAS),
        "nab_r": np.asarray(-ab_tr, NPBF16).reshape(1, D_GAS),
        "hb_r": np.asarray(h_bias, NPBF16).reshape(1, D_GAS),
        # per-partition scalars for the n=8 descending df
        "ne9": np.ascontiguousarray(
            (-e9).astype(f32).reshape(2, 128).T
        ),  # [128, 2]
    }


def build_nc():
    nc = bacc.Bacc("TRN2", target_bir_lowering=False, debug=False)

    x_d = nc.dram_tensor("x", (ROWS, D_IN), BF16, kind="ExternalInput").ap()
    wge_d = nc.dram_tensor("wge", (128, 4, D_GAS), BF16, kind="ExternalInput").ap()
    wdesc_d = nc.dram_tensor("wdesc", (128, 2, D_GAS), BF16, kind="ExternalInput").ap()
    wasc_d = nc.dram_tensor("wasc", (128, 2, D_GAS), BF16, kind="ExternalInput").ap()
    wfold_d = nc.dram_tensor("wfold", (128, 2, D_GAS), BF16, kind="ExternalInput").ap()
    w1g_d = nc.dram_tensor("w1g", (128, 2, D_GAS), BF16, kind="ExternalInput").ap()
    w2_d = nc.dram_tensor("w2", (128, 2, D_OUT), BF16, kind="ExternalInput").ap()
    iden_d = nc.dram_tensor("iden", (128, 128), BF16, kind="ExternalInput").ap()
    ones_d = nc.dram_tensor("ones", (1, R), BF16, kind="ExternalInput").ap()
    bge_d = nc.dram_tensor("bge_r", (1, D_GAS), BF16, kind="ExternalInput").ap()
    be8_d = nc.dram_tensor("be8_r", (1, D_GAS), BF16, kind="ExternalInput").ap()
    c3_d = nc.dram_tensor("c3_r", (1, D_GAS), BF16, kind="ExternalInput").ap()
    nab_d = nc.dram_tensor("nab_r", (1, D_GAS), BF16, kind="ExternalInput").ap()
    hb_d = nc.dram_tensor("hb_r", (1, D_GAS), BF16, kind="ExternalInput").ap()
    ne9_d = nc.dram_tensor("ne9", (128, 2), F32, kind="ExternalInput").ap()
    out_d = nc.dram_tensor("out", (ROWS, D_OUT), F32, kind="ExternalOutput").ap()

    with tile.TileContext(nc) as tc:
        with (
            tc.tile_pool(name="const", bufs=1) as cpool,
            tc.tile_pool(name="state", bufs=2) as spool,
            tc.tile_pool(name="work", bufs=3) as wpool,
            tc.tile_pool(name="psum", bufs=1, space="PSUM") as ppool,
        ):
            # ---- constants ----
            wge_t = cpool.tile([128, 4, D_GAS], BF16, tag="wge")
            nc.sync.dma_start(wge_t, wge_d)
            wdesc_t = cpool.tile([128, 2, D_GAS], BF16, tag="wdesc")
            nc.sync.dma_start(wdesc_t, wdesc_d)
            wasc_t = cpool.tile([128, 2, D_GAS], BF16, tag="wasc")
            nc.sync.dma_start(wasc_t, wasc_d)
            wfold_t = cpool.tile([128, 2, D_GAS], BF16, tag="wfold")
            nc.sync.dma_start(wfold_t, wfold_d)
            w1g_t = cpool.tile([128, 2, D_GAS], BF16, tag="w1g")
            nc.sync.dma_start(w1g_t, w1g_d)
            w2_t = cpool.tile([128, 2, D_OUT], BF16, tag="w2")
            nc.sync.dma_start(w2_t, w2_d)
            iden_t = cpool.tile([128, 128], BF16, tag="iden")
            nc.sync.dma_start(iden_t, iden_d)
            ones_t = cpool.tile([1, R], BF16, tag="ones")
            nc.sync.dma_start(ones_t, ones_d)
            bge_t = cpool.tile([1, D_GAS], BF16, tag="bge")
            nc.sync.dma_start(bge_t, bge_d)
            be8_t = cpool.tile([1, D_GAS], BF16, tag="be8")
            nc.sync.dma_start(be8_t, be8_d)
            c3_t = cpool.tile([1, D_GAS], BF16, tag="c3")
            nc.sync.dma_start(c3_t, c3_d)
            nab_t = cpool.tile([1, D_GAS], BF16, tag="nab")
            nc.sync.dma_start(nab_t, nab_d)
            hb_t = cpool.tile([1, D_GAS], BF16, tag="hb")
            nc.sync.dma_start(hb_t, hb_d)
            ne9_t = cpool.tile([128, 2], F32, tag="ne9")
            nc.sync.dma_start(ne9_t, ne9_d)

            def bias_mm(acc, row_t, last=False):
                """Add a [1,256] bias row to both ft halves of a [128,2,R]
                PSUM accumulator via K=1 ones-matmuls."""
                for ft in range(2):
                    nc.tensor.matmul(
                        acc[:, ft, :],
                        lhsT=row_t[0:1, ft * 128 : (ft + 1) * 128],
                        rhs=ones_t[0:1, :],
                        start=False,
                        stop=last,
                        skip_group_check=True,
                    )

            def wmm(acc, w_t, rhs, start=False, stop=False):
                """acc[128,2,R] += rhs @ W  (W given k-major [128,2,256])."""
                for ft in range(2):
                    for k in range(2):
                        nc.tensor.matmul(
                            acc[:, ft, :],
                            lhsT=w_t[:, k, ft * 128 : (ft + 1) * 128],
                            rhs=rhs[:, k, :],
                            start=start and k == 0,
                            stop=stop and k == 1,
                            skip_group_check=not (start and k == 0),
                        )

            def iden_mm(acc, rhs, start=False, stop=False):
                """acc[128,2,R] += rhs (injected through the PE identity)."""
                for ft in range(2):
                    nc.tensor.matmul(
                        acc[:, ft, :],
                        lhsT=iden_t,
                        rhs=rhs[:, ft, :],
                        start=start,
                        stop=stop,
                        skip_group_check=not start,
                    )

            def chunk_gen(c):
                p = c % 2
                r0 = c * R

                # ---- x loaded transposed by the DMA xbar ----
                xT = spool.tile([128, 4, R], BF16, tag=f"xT{p}")
                nc.sync.dma_start_transpose(xT, x_d[r0 : r0 + R, :])

                acc = ppool.tile([128, 2, R], F32, tag=f"acc{p}", name=f"acc{c}")

                # ---- encoder: g0 = relu(x @ W_ge + b_ge) ----
                for ft in range(2):
                    for k in range(4):
                        nc.tensor.matmul(
                            acc[:, ft, :],
                            lhsT=wge_t[:, k, ft * 128 : (ft + 1) * 128],
                            rhs=xT[:, k, :],
                            start=(k == 0),
                            stop=False,
                        )
                bias_mm(acc, bge_t, last=True)
                g0 = spool.tile([128, 2, R], BF16, tag=f"g0{p}")
                nc.scalar.activation(g0, acc, AF.Relu)
                yield

                st = {0: g0}
                S = None
                for sweep in range(2):
                    last = sweep == 1
                    # ---------- descending sweep ----------
                    for n in range(N_PLATES, 0, -1):
                        df = wpool.tile([128, 2, R], BF16, tag=f"df{p}", bufs=3)
                        g_prev = st[n - 1] if (last and n > 1) else g0
                        if n == N_PLATES:
                            for ft in range(2):
                                nc.vector.tensor_scalar(
                                    df[:, ft, :], g_prev[:, ft, :],
                                    ne9_t[:, ft : ft + 1], None, OP.add,
                                )
                        else:
                            nc.vector.tensor_tensor(df, g_prev, st[n + 1], OP.subtract)
                        if last:
                            if n == N_PLATES:
                                S = spool.tile([128, 2, R], BF16, tag=f"S{p}")
                                nc.gpsimd.tensor_copy(S, df)
                            else:
                                nc.gpsimd.tensor_tensor(S, S, df, OP.add)
                        wmm(acc, wdesc_t, df, start=(n == N_PLATES))
                        bias_mm(acc, be8_t if n == N_PLATES else c3_t, last=True)
                        e_new = spool.tile([128, 2, R], BF16, tag=f"st{p}_{n}")
                        nc.scalar.activation(e_new, acc, AF.Sigmoid)
                        st[n] = e_new
                        yield

                    # ---------- ascending sweep ----------
                    if not last:
                        # materialize g_1..g_7 into SBUF for the next
                        # descending sweep (g_8 of sweep 0 is dead)
                        for n in range(1, N_PLATES):
                            df = wpool.tile([128, 2, R], BF16, tag=f"df{p}", bufs=3)
                            nc.vector.tensor_tensor(df, st[n - 1], st[n], OP.subtract)
                            use_act = n in ASC0_ACT_PLATES
                            wmm(acc, wasc_t, df, start=True)
                            if use_act:
                                iden_mm(acc, st[n - 1])
                            bias_mm(acc, nab_t, last=True)
                            g_new = spool.tile([128, 2, R], BF16, tag=f"st{p}_{n}")
                            if use_act:
                                nc.scalar.activation(g_new, acc, AF.Copy)
                            else:
                                nc.vector.tensor_tensor(g_new, st[n - 1], acc, OP.add)
                            st[n] = g_new
                            yield
                    else:
                        # keep g in the PSUM accumulator; only g_8 leaves
                        for n in range(1, N_PLATES + 1):
                            df = wpool.tile([128, 2, R], BF16, tag=f"df{p}", bufs=3)
                            if n == 1:
                                nc.vector.tensor_tensor(df, g0, st[1], OP.subtract)
                                iden_mm(acc, g0, start=True)
                            else:
                                # GPSIMD cannot read PSUM; DVE does these
                                nc.vector.tensor_tensor(df, acc, st[n], OP.subtract)
                            wmm(acc, wasc_t, df)
                            bias_mm(acc, nab_t, last=True)
                            yield
                        g8 = spool.tile([128, 2, R], BF16, tag=f"st{p}_8")
                        nc.scalar.activation(g8, acc, AF.Copy)
                        st[N_PLATES] = g8
                        yield

                # ---------- head ----------
                wmm(acc, w1g_t, st[N_PLATES], start=True)
                wmm(acc, wfold_t, S)
                bias_mm(acc, hb_t, last=True)
                h = spool.tile([128, 2, R], BF16, tag=f"h{p}")
                nc.scalar.activation(h, acc, AF.Relu)
                yield

                for rb in range(R // 128):
                    po = ppool.tile([128, D_OUT], F32, tag="po", bufs=2)
                    for n0, nw in ((0, 512), (512, 488)):
                        for ft in range(2):
                            nc.tensor.matmul(
                                po[:, n0 : n0 + nw],
                                lhsT=h[:, ft, rb * 128 : (rb + 1) * 128],
                                rhs=w2_t[:, ft, n0 : n0 + nw],
                                start=(ft == 0),
                                stop=(ft == 1),
                            )
                    stage = wpool.tile([128, D_OUT], F32, tag="stage", bufs=3)
                    nc.scalar.activation(stage, po, AF.Copy)
                    nc.sync.dma_start(
                        out_d[r0 + rb * 128 : r0 + (rb + 1) * 128, :], stage
                    )
                    yield

            # interleave issue order within each chunk pair
            for pair in ((0, 1), (2, 3)):
                gens = [chunk_gen(c) for c in pair]
                alive = list(gens)
                while alive:
                    for g in list(alive):
                        try:
                            next(g)
                        except StopIteration:
                            alive.remove(g)

    nc.compile()
    return nc


_NC_CACHE = {}


def _get_nc():
    if "nc" not in _NC_CACHE:
        _NC_CACHE["nc"] = build_nc()
    return _NC_CACHE["nc"]


def run_hw(inputs, trace=False):
    inp = {k: np.asarray(v) for k, v in inputs.items()}
    prep = _preprocess_weights(inp)
    x = np.asarray(inp["x"], dtype=np.float32).astype(NPBF16)
    b2 = np.asarray(inp["b2"], np.float32)

    nc = _get_nc()
    in_maps = []
    for c in range(N_CORES):
        m = {"x": np.ascontiguousarray(x[c * ROWS : (c + 1) * ROWS])}
        m.update(prep)
        in_maps.append(m)
    res = bass_utils.run_bass_kernel_spmd(
        nc, in_maps, core_ids=list(range(N_CORES)), trace=trace
    )
    out = np.concatenate([res.results[c]["out"] for c in range(N_CORES)], axis=0)
    out = out + b2.reshape(1, D_OUT)
    return out, res


def kernel(**inputs):
    out, _ = run_hw(inputs, trace=False)
    return out


# revision 6
# speedup vs baseline: 2.4915x; 1.3207x over previous
"""Trainium2 Bass kernel for nn_CounterFlowNetwork.

Data-parallel over 8 NeuronCores (batch sharded).  v3: on top of the v2
restructure (bf16 everywhere, DMA-xbar transposed x load, PSUM-resident
accumulators, chunk-pair issue interleaving), ALL K=1 bias-injection
matmuls are gone:

 - The per-plate ascending bias -alpha*b_tr is simply not applied on
   device.  The stored gas state drifts by a host-computable constant
   delta_n = delta_{n-1}(I - alpha W_tr) + alpha b_tr per ascending
   plate; the drift is corrected in the descending sigmoid bias table
   (per sweep/plate/ft), in the head bias (for g_8 and for S, the
   driving-force sum), all folded host-side in float64.
 - Descending sigmoid biases (b_eq + (9-n)c3 + drift correction) ride
   ACT's per-partition bias port with per-ft activations instead of
   ones-matmuls into PSUM.
 - Encoder/head ReLU biases likewise.
 - Output bias b2 is added host-side after the gather.

This removes ~260 N=512 matmuls per core (~30% of tensor-engine time in
v2, which profiled at 85% busy).
"""

import numpy as np

import concourse.bass as bass
import concourse.bacc as bacc
import concourse.mybir as mybir
import concourse.tile as tile
from concourse import bass_utils

B, D_IN, D_GAS, D_OUT = 16384, 512, 256, 1000
N_PLATES = 8
N_CORES = 8
ROWS = B // N_CORES          # rows per core
N_CHUNKS = 4
R = ROWS // N_CHUNKS         # rows per chunk
F32 = mybir.dt.float32
BF16 = mybir.dt.bfloat16
NPBF16 = mybir.dt.np(BF16)
AF = mybir.ActivationFunctionType
OP = mybir.AluOpType


def _preprocess_weights(inp):
    """Fold the plate linear algebra host-side (float64, cast to bf16)."""
    f32, f64 = np.float32, np.float64
    W_tr = np.asarray(inp["W_tr"], f64)
    b_tr = np.asarray(inp["b_tr"], f64)
    W_ab = np.asarray(inp["W_ab"], f64)
    b_ab = np.asarray(inp["b_ab"], f64)
    W_eq = np.asarray(inp["W_eq"], f64)
    b_eq = np.asarray(inp["b_eq"], f64)
    W1 = np.asarray(inp["W1"], f64)
    b1 = np.asarray(inp["b1"], f64)
    alpha = float(np.asarray(inp["alpha"]))

    W_trp = alpha * W_tr
    ab_tr = alpha * b_tr
    W_trab = W_trp @ W_ab
    c2 = ab_tr @ W_ab + b_ab
    W_trabeq = W_trab @ W_eq
    c3 = c2 @ W_eq
    W1_g, W1_l = W1[:D_GAS], W1[D_GAS:]
    W_fold = W_trab @ W1_l
    e9 = 1.0 / (1.0 + np.exp(-b_eq))
    h_bias = b1 + 8.0 * (c2 @ W1_l)

    # ascending-bias deferral: stored g after n ascending plates is
    # g_true + delta_n with delta_n = delta_{n-1} @ (I - aW_tr) + a b_tr
    M = np.eye(D_GAS) - W_trp
    delta = [np.zeros(D_GAS)]
    for _ in range(N_PLATES):
        delta.append(delta[-1] @ M + ab_tr)
    # descending sigmoid bias, per sweep s and plate n:
    #   b_eq + (9-n) c3 - [s==1] * (sum_{m=n..8} delta_{m-1}) @ W_trabeq
    bias_tab = np.zeros((2, N_PLATES, D_GAS))
    for n in range(1, N_PLATES + 1):
        dsum = np.sum(delta[n - 1 : N_PLATES], axis=0)  # delta_{n-1}..delta_7
        bias_tab[0, n - 1] = b_eq + (9 - n) * c3
        bias_tab[1, n - 1] = b_eq + (9 - n) * c3 - dsum @ W_trabeq
    # head: g8 drifts by delta_8, S drifts by T = sum_{m=0..7} delta_m
    T = np.sum(delta[0:N_PLATES], axis=0)
    h_bias = h_bias - delta[N_PLATES] @ W1_g - T @ W_fold

    def kmaj(w):  # (K, M) -> [128, K//128, M] partition-major contraction
        w = np.asarray(w, NPBF16)
        k, m = w.shape
        return np.ascontiguousarray(w.reshape(k // 128, 128, m).transpose(1, 0, 2))

    def pscal(v):  # (256,) -> [128, 2] per-partition scalars (f32)
        return np.ascontiguousarray(np.asarray(v, f32).reshape(2, 128).T)

    return {
        "wge": kmaj(np.asarray(inp["W_ge"], f32)),
        "wdesc": kmaj(W_trabeq),
        "wasc": kmaj(-W_trp),
        "wfold": kmaj(W_fold),
        "w1g": kmaj(W1_g),
        "w2": kmaj(np.asarray(inp["W2"], f32)),
        "iden": np.eye(128, dtype=np.float32).astype(NPBF16),
        "bge": pscal(np.asarray(inp["b_ge"], f32)),
        "hb": pscal(h_bias),
        "ne9": pscal(-e9),
        # [128, sweep, plate, ft]
        "bes": np.ascontiguousarray(
            bias_tab.reshape(2, N_PLATES, 2, 128).transpose(3, 0, 1, 2).astype(f32)
        ),
    }


def build_nc():
    nc = bacc.Bacc("TRN2", target_bir_lowering=False, debug=False)

    x_d = nc.dram_tensor("x", (ROWS, D_IN), BF16, kind="ExternalInput").ap()
    wge_d = nc.dram_tensor("wge", (128, 4, D_GAS), BF16, kind="ExternalInput").ap()
    wdesc_d = nc.dram_tensor("wdesc", (128, 2, D_GAS), BF16, kind="ExternalInput").ap()
    wasc_d = nc.dram_tensor("wasc", (128, 2, D_GAS), BF16, kind="ExternalInput").ap()
    wfold_d = nc.dram_tensor("wfold", (128, 2, D_GAS), BF16, kind="ExternalInput").ap()
    w1g_d = nc.dram_tensor("w1g", (128, 2, D_GAS), BF16, kind="ExternalInput").ap()
    w2_d = nc.dram_tensor("w2", (128, 2, D_OUT), BF16, kind="ExternalInput").ap()
    iden_d = nc.dram_tensor("iden", (128, 128), BF16, kind="ExternalInput").ap()
    bge_d = nc.dram_tensor("bge", (128, 2), F32, kind="ExternalInput").ap()
    hb_d = nc.dram_tensor("hb", (128, 2), F32, kind="ExternalInput").ap()
    ne9_d = nc.dram_tensor("ne9", (128, 2), F32, kind="ExternalInput").ap()
    bes_d = nc.dram_tensor("bes", (128, 2, N_PLATES, 2), F32, kind="ExternalInput").ap()
    out_d = nc.dram_tensor("out", (ROWS, D_OUT), F32, kind="ExternalOutput").ap()

    with tile.TileContext(nc) as tc:
        with (
            tc.tile_pool(name="const", bufs=1) as cpool,
            tc.tile_pool(name="state", bufs=2) as spool,
            tc.tile_pool(name="work", bufs=3) as wpool,
            tc.tile_pool(name="psum", bufs=1, space="PSUM") as ppool,
        ):
            # ---- constants ----
            wge_t = cpool.tile([128, 4, D_GAS], BF16, tag="wge")
            nc.sync.dma_start(wge_t, wge_d)
            wdesc_t = cpool.tile([128, 2, D_GAS], BF16, tag="wdesc")
            nc.sync.dma_start(wdesc_t, wdesc_d)
            wasc_t = cpool.tile([128, 2, D_GAS], BF16, tag="wasc")
            nc.sync.dma_start(wasc_t, wasc_d)
            wfold_t = cpool.tile([128, 2, D_GAS], BF16, tag="wfold")
            nc.sync.dma_start(wfold_t, wfold_d)
            w1g_t = cpool.tile([128, 2, D_GAS], BF16, tag="w1g")
            nc.sync.dma_start(w1g_t, w1g_d)
            w2_t = cpool.tile([128, 2, D_OUT], BF16, tag="w2")
            nc.sync.dma_start(w2_t, w2_d)
            iden_t = cpool.tile([128, 128], BF16, tag="iden")
            nc.sync.dma_start(iden_t, iden_d)
            bge_t = cpool.tile([128, 2], F32, tag="bge")
            nc.sync.dma_start(bge_t, bge_d)
            hb_t = cpool.tile([128, 2], F32, tag="hb")
            nc.sync.dma_start(hb_t, hb_d)
            ne9_t = cpool.tile([128, 2], F32, tag="ne9")
            nc.sync.dma_start(ne9_t, ne9_d)
            bes_t = cpool.tile([128, 2, N_PLATES, 2], F32, tag="bes")
            nc.sync.dma_start(bes_t, bes_d)

            def wmm(acc, w_t, rhs, start=False, stop=False):
                """acc[128,2,R] += rhs @ W  (W given k-major [128,2,256])."""
                for ft in range(2):
                    for k in range(2):
                        nc.tensor.matmul(
                            acc[:, ft, :],
                            lhsT=w_t[:, k, ft * 128 : (ft + 1) * 128],
                            rhs=rhs[:, k, :],
                            start=start and k == 0,
                            stop=stop and k == 1,
                            skip_group_check=not (start and k == 0),
                        )

            def iden_mm(acc, rhs, start=False, stop=False):
                """acc[128,2,R] += rhs (injected through the PE identity)."""
                for ft in range(2):
                    nc.tensor.matmul(
                        acc[:, ft, :],
                        lhsT=iden_t,
                        rhs=rhs[:, ft, :],
                        start=start,
                        stop=stop,
                        skip_group_check=not start,
                    )

            def chunk_gen(c):
                p = c % 2
                r0 = c * R

                # ---- x loaded transposed by the DMA xbar ----
                xT = spool.tile([128, 4, R], BF16, tag=f"xT{p}")
                nc.sync.dma_start_transpose(xT, x_d[r0 : r0 + R, :])

                acc = ppool.tile([128, 2, R], F32, tag=f"acc{p}", name=f"acc{c}")

                # ---- encoder: g0 = relu(x @ W_ge + b_ge) ----
                for ft in range(2):
                    for k in range(4):
                        nc.tensor.matmul(
                            acc[:, ft, :],
                            lhsT=wge_t[:, k, ft * 128 : (ft + 1) * 128],
                            rhs=xT[:, k, :],
                            start=(k == 0),
                            stop=(k == 3),
                        )
                g0 = spool.tile([128, 2, R], BF16, tag=f"g0{p}")
                for ft in range(2):
                    nc.scalar.activation(
                        g0[:, ft, :], acc[:, ft, :], AF.Relu,
                        bias=bge_t[:, ft : ft + 1],
                    )
                yield

                st = {0: g0}
                S = None
                for sweep in range(2):
                    last = sweep == 1
                    # ---------- descending sweep ----------
                    for n in range(N_PLATES, 0, -1):
                        df = wpool.tile([128, 2, R], BF16, tag=f"df{p}", bufs=3)
                        g_prev = st[n - 1] if (last and n > 1) else g0
                        if n == N_PLATES:
                            for ft in range(2):
                                nc.vector.tensor_scalar(
                                    df[:, ft, :], g_prev[:, ft, :],
                                    ne9_t[:, ft : ft + 1], None, OP.add,
                                )
                        else:
                            nc.vector.tensor_tensor(df, g_prev, st[n + 1], OP.subtract)
                        if last:
                            if n == N_PLATES:
                                S = spool.tile([128, 2, R], BF16, tag=f"S{p}")
                                nc.gpsimd.tensor_copy(S, df)
                            else:
                                nc.gpsimd.tensor_tensor(S, S, df, OP.add)
                        wmm(acc, wdesc_t, df, start=(n == N_PLATES), stop=True)
                        e_new = spool.tile([128, 2, R], BF16, tag=f"st{p}_{n}")
                        for ft in range(2):
                            nc.scalar.activation(
                                e_new[:, ft, :], acc[:, ft, :], AF.Sigmoid,
                                bias=bes_t[:, sweep, n - 1, ft : ft + 1],
                            )
                        st[n] = e_new
                        yield

                    # ---------- ascending sweep ----------
                    if not last:
                        # materialize g_1..g_7 into SBUF for the next
                        # descending sweep (g_8 of sweep 0 is dead)
                        for n in range(1, N_PLATES):
                            df = wpool.tile([128, 2, R], BF16, tag=f"df{p}", bufs=3)
                            nc.vector.tensor_tensor(df, st[n - 1], st[n], OP.subtract)
                            wmm(acc, wasc_t, df, start=True, stop=True)
                            g_new = spool.tile([128, 2, R], BF16, tag=f"st{p}_{n}")
                            nc.vector.tensor_tensor(g_new, st[n - 1], acc, OP.add)
                            st[n] = g_new
                            yield
                    else:
                        # keep g in the PSUM accumulator; only g_8 leaves
                        for n in range(1, N_PLATES + 1):
                            df = wpool.tile([128, 2, R], BF16, tag=f"df{p}", bufs=3)
                            if n == 1:
                                nc.vector.tensor_tensor(df, g0, st[1], OP.subtract)
                                iden_mm(acc, g0, start=True)
                            else:
                                # GPSIMD cannot read PSUM; DVE does these
                                nc.vector.tensor_tensor(df, acc, st[n], OP.subtract)
                            wmm(acc, wasc_t, df, stop=True)
                            yield
                        g8 = spool.tile([128, 2, R], BF16, tag=f"st{p}_8")
                        nc.scalar.activation(g8, acc, AF.Copy)
                        st[N_PLATES] = g8
                        yield

                # ---------- head ----------
                wmm(acc, w1g_t, st[N_PLATES], start=True)
                wmm(acc, wfold_t, S, stop=True)
                h = spool.tile([128, 2, R], BF16, tag=f"h{p}")
                for ft in range(2):
                    nc.scalar.activation(
                        h[:, ft, :], acc[:, ft, :], AF.Relu,
                        bias=hb_t[:, ft : ft + 1],
                    )
                yield

                for rb in range(R // 128):
                    po = ppool.tile([128, D_OUT], F32, tag="po", bufs=2)
                    for n0, nw in ((0, 512), (512, 488)):
                        for ft in range(2):
                            nc.tensor.matmul(
                                po[:, n0 : n0 + nw],
                                lhsT=h[:, ft, rb * 128 : (rb + 1) * 128],
                                rhs=w2_t[:, ft, n0 : n0 + nw],
                                start=(ft == 0),
                                stop=(ft == 1),
                            )
                    stage = wpool.tile([128, D_OUT], F32, tag="stage", bufs=3)
                    nc.scalar.activation(stage, po, AF.Copy)
                    nc.sync.dma_start(
                        out_d[r0 + rb * 128 : r0 + (rb + 1) * 128, :], stage
                    )
                    yield

            # interleave issue order within each chunk pair
            for pair in ((0, 1), (2, 3)):
                gens = [chunk_gen(c) for c in pair]
                alive = list(gens)
                while alive:
                    for g in list(alive):
                        try:
                            next(g)
                        except StopIteration:
                            alive.remove(g)

    nc.compile()
    return nc


_NC_CACHE = {}


def _get_nc():
    if "nc" not in _NC_CACHE:
        _NC_CACHE["nc"] = build_nc()
    return _NC_CACHE["nc"]


def run_hw(inputs, trace=False):
    inp = {k: np.asarray(v) for k, v in inputs.items()}
    prep = _preprocess_weights(inp)
    x = np.asarray(inp["x"], dtype=np.float32).astype(NPBF16)
    b2 = np.asarray(inp["b2"], np.float32)

    nc = _get_nc()
    in_maps = []
    for c in range(N_CORES):
        m = {"x": np.ascontiguousarray(x[c * ROWS : (c + 1) * ROWS])}
        m.update(prep)
        in_maps.append(m)
    res = bass_utils.run_bass_kernel_spmd(
        nc, in_maps, core_ids=list(range(N_CORES)), trace=trace
    )
    out = np.concatenate([res.results[c]["out"] for c in range(N_CORES)], axis=0)
    out = out + b2.reshape(1, D_OUT)
    return out, res


def kernel(**inputs):
    out, _ = run_hw(inputs, trace=False)
    return out
